# revision 2
# baseline (speedup 1.0000x reference)
"""Multi-Head Latent Attention (MLA) prefill kernel for 8 Trainium2 NeuronCores.

v2: bf16 compute/collectives, split AllGather (kv first, q second) overlapped
with up-projections, interleaved up-proj + attention for a continuous PE
stream, consolidated weight DMAs, wo prefetched during attention.

Sharding: latent down-projections row-split 8 ways + AllGather; up-projections
and attention head-split (2 heads/core); AllToAll converts head-split attention
output to token-split for the output projection.
"""
import sys
if '/opt/trn_rl_repo' not in sys.path:
    sys.path.insert(0, '/opt/trn_rl_repo')

import math
from contextlib import ExitStack
import numpy as np
import ml_dtypes

import concourse.bass as bass
import concourse.tile as tile
import concourse.mybir as mybir
from concourse import bacc

F32 = mybir.dt.float32
F32R = mybir.dt.float32r
BF16 = mybir.dt.bfloat16
AF = mybir.ActivationFunctionType
ALU = mybir.AluOpType
BF = ml_dtypes.bfloat16

B, S, DIM, H = 2, 2048, 2048, 16
NOPE, ROPE, QKD, VD = 128, 64, 192, 128
QLR, KVLR = 512, 512
EPS = 1e-6
NC = 8
N = B * S              # 4096 flattened tokens
R = N // NC            # 512 tokens per core
HPC = H // NC          # 2 heads per core
NBLK = N // R          # 8 token blocks
SCALE = 1.0 / math.sqrt(QKD)
KVC = 5                # kv-latent chunks in AG payload (4 latent + rope)
SKIP, PLAIN = -2, -1


def _rope_tables():
    freqs = (1.0 / (10000.0 ** (np.arange(0, ROPE, 2, dtype=np.float32) / ROPE)))
    ang = np.arange(S, dtype=np.float32)[:, None] * freqs[None, :]      # [S, 32]
    return np.cos(ang).T.copy(), np.sin(ang).T.copy()                   # [32, S]


def _classify_mask(mask):
    """Per (q-chunk of 512, k-block of 128): SKIP / PLAIN / index into deduped
    transposed mask blocks [128 k, 512 q]."""
    cls = [[PLAIN] * (S // 128) for _ in range(S // 512)]
    blocks, keys = [], {}
    for qc in range(S // 512):
        sub_q = mask[qc * 512:(qc + 1) * 512]
        for kb in range(S // 128):
            blk = sub_q[:, kb * 128:(kb + 1) * 128]
            if not blk.any():
                cls[qc][kb] = SKIP
            elif blk.all():
                cls[qc][kb] = PLAIN
            else:
                key = blk.tobytes()
                if key not in keys:
                    keys[key] = len(blocks)
                    blocks.append(blk.T.astype(BF))   # [128 k, 512 q]
                cls[qc][kb] = keys[key]
    blocks = (np.stack(blocks) if blocks
              else np.zeros((1, 128, 512), BF))
    return cls, blocks


def _build(cls, nmask, flags, repeat=1, sim_mode=False):
    """Emit the bass program. cls/nmask/flags are compile-time schedule data."""
    nc = bacc.Bacc(None, num_devices=NC)

    # ---- I/O ----
    # x, core slab, token-major: [128, 16*R], cols k*R.. = dim chunk k
    x_c = nc.dram_tensor("x_c", [128, (DIM // 128) * R], BF16, kind="ExternalInput")
    # down-proj weights, lhsT layout: [128, nm*16*128], cols (m*16+k)*128..
    wqaT = nc.dram_tensor("wqaT", [128, 4 * 16 * 128], BF16, kind="ExternalInput")
    wkvaT = nc.dram_tensor("wkvaT", [128, 5 * 16 * 128], BF16, kind="ExternalInput")
    bqa = nc.dram_tensor("bqa", [QLR], F32, kind="ExternalInput")
    bkva = nc.dram_tensor("bkva", [KVLR + ROPE], F32, kind="ExternalInput")
    normw = nc.dram_tensor("normw", [1, QLR + KVLR], F32, kind="ExternalInput")
    # phase-1 trig (this core's R positions): [c;c] and [s;s] halves, 64 rows
    trig1c = nc.dram_tensor("trig1c", [64, R], F32, kind="ExternalInput")   # [c;c]
    trig1s = nc.dram_tensor("trig1s", [64, R], F32, kind="ExternalInput")   # [s;s]
    # phase-2 trig (full S positions): [c;c;c;c], [s;s;s;s], 128 rows
    trigqc = nc.dram_tensor("trigqc", [128, S], F32, kind="ExternalInput")
    trigqs = nc.dram_tensor("trigqs", [128, S], F32, kind="ExternalInput")
    sgn = nc.dram_tensor("sgn", [128, 1], F32, kind="ExternalInput")
    # up-proj weights, lhsT layouts
    wqbT = nc.dram_tensor("wqbT", [128, 3 * 4 * 128], BF16, kind="ExternalInput")
    wkbT = nc.dram_tensor("wkbT", [128, 2 * 4 * 128], BF16, kind="ExternalInput")
    wvbT = nc.dram_tensor("wvbT", [128, 4 * 256], BF16, kind="ExternalInput")
    bqb = nc.dram_tensor("bqb", [HPC * QKD], F32, kind="ExternalInput")
    bkb = nc.dram_tensor("bkb", [HPC * NOPE], F32, kind="ExternalInput")
    bvb = nc.dram_tensor("bvb", [1, HPC * VD], BF16, kind="ExternalInput")
    # output projection, moving-tensor layout: [128, 4n*16k*512]
    woT = nc.dram_tensor("woT", [128, 4 * 16 * 512], BF16, kind="ExternalInput")
    wob = nc.dram_tensor("wob", [1, DIM], BF16, kind="ExternalInput")
    maskblk = nc.dram_tensor("maskblk", [max(nmask, 1), 128, 512], BF16,
                             kind="ExternalInput")
    out_c = nc.dram_tensor("out", [R, DIM], F32, kind="ExternalOutput")

    with tile.TileContext(nc) as tc:
        with tc.tile_pool(name="konst", bufs=1) as konst, \
             tc.tile_pool(name="dram", bufs=1, space="DRAM") as dram:

            # ---- constants ----
            ones_f = konst.tile([128, 1], F32)
            nc.vector.memset(ones_f[:, :], 1.0)
            ones_col = konst.tile([128, 1], F32R)
            nc.vector.tensor_copy(out=ones_col[:, :], in_=ones_f[:, :])
            ones_col_b = konst.tile([128, 1], BF16)
            nc.vector.memset(ones_col_b[:, :], 1.0)
            ones_rf = konst.tile([1, 128], F32)
            nc.vector.memset(ones_rf[:, :], 1.0)
            ones_row = konst.tile([1, 128], F32R)
            nc.vector.tensor_copy(out=ones_row[:, :], in_=ones_rf[:, :])
            if flags['bvb'] or flags['wob']:
                ones_row_b = konst.tile([1, 128], BF16)
                nc.vector.memset(ones_row_b[:, :], 1.0)
            sgn_t = konst.tile([128, 1], F32)
            nc.sync.dma_start(out=sgn_t, in_=sgn[:, :])
            eps_t = konst.tile([1, 1], F32)
            nc.vector.memset(eps_t[:, :], EPS)

            for _rep in range(repeat):
                # ---- collective DRAM tiles ----
                agkv_in = dram.tile([128, KVC * R], BF16, name=f"agkvi{_rep}")
                agkv_out = dram.tile([NC, 128, KVC * R], BF16,
                                     addr_space=("Local" if sim_mode else "Shared"),
                                     name=f"agkvo{_rep}")
                agq_in = dram.tile([128, 4 * R], BF16, name=f"agqi{_rep}")
                agq_out = dram.tile([NC, 128, 4 * R], BF16,
                                    addr_space=("Local" if sim_mode else "Shared"),
                                    name=f"agqo{_rep}")
                a2a_in = dram.tile([NC, HPC * VD, R], BF16, name=f"a2ai{_rep}")
                a2a_out = dram.tile([NC, HPC * VD, R], BF16, name=f"a2ao{_rep}")

                # ================= PHASE 1: latent down-proj (row shard) =========
                with tc.tile_pool(name=f"p1sb{_rep}", bufs=1) as p1sb, \
                     tc.tile_pool(name=f"p1a{_rep}", bufs=1) as p1a, \
                     tc.tile_pool(name=f"p1tmp{_rep}", bufs=2) as p1tmp, \
                     tc.tile_pool(name=f"p1ps{_rep}", bufs=3, space="PSUM") as p1ps, \
                     tc.tile_pool(name=f"p1ps2{_rep}", bufs=2, space="PSUM") as p1ps2, \
                     tc.tile_pool(name=f"p1ps1{_rep}", bufs=1, space="PSUM") as p1ps1:

                    xt = p1sb.tile([128, (DIM // 128) * R], BF16)
                    nc.sync.dma_start(out=xt, in_=x_c[:, :])
                    wqa_t = p1sb.tile([128, 4 * 16 * 128], BF16)
                    nc.sync.dma_start(out=wqa_t, in_=wqaT[:, :])
                    wkva_t = p1sb.tile([128, 5 * 16 * 128], BF16)
                    nc.sync.dma_start(out=wkva_t, in_=wkvaT[:, :])
                    t1c = p1sb.tile([64, R], F32)
                    nc.sync.dma_start(out=t1c, in_=trig1c[:, :])
                    t1s = p1sb.tile([64, R], F32)
                    nc.sync.dma_start(out=t1s, in_=trig1s[:, :])
                    nw_t = p1sb.tile([1, QLR + KVLR], F32R)
                    nc.sync.dma_start(out=nw_t, in_=normw[:, :].bitcast(F32R))

                    stages = {}
                    for path in (1, 0):   # kv first (its AG gates phase 2 start)
                        wt, nm = (wkva_t, 5) if path == 1 else (wqa_t, 4)
                        stage = p1a.tile([128, nm * R], BF16, name=f"stage{path}")
                        stages[path] = stage
                        if path == 1:
                            # rope chunk occupies rows 0:64 of the last R-col
                            # block; zero rows 64:128 (gathered but unused)
                            nc.vector.memset(stage[64:128, 4 * R:5 * R], 0.0)
                        acts = []
                        sums_ps = p1ps1.tile([1, R], F32, name=f"sums{path}",
                                             tag="sums")
                        for m in range(nm):
                            mp = 64 if m == 4 else 128
                            ps = p1ps.tile([128, R], F32, tag="p1acc")
                            for k in range(DIM // 128):
                                nc.tensor.matmul(
                                    ps[:, :],
                                    wt[:, (m * 16 + k) * 128:(m * 16 + k + 1) * 128],
                                    xt[:, k * R:(k + 1) * R],
                                    start=(k == 0), stop=(k == DIM // 128 - 1))
                            a = p1a.tile([128, R], F32, tag=f"act{m}",
                                         name=f"a{path}{m}")
                            if flags['ba'][path]:
                                bias_d = bkva if path == 1 else bqa
                                bt = p1sb.tile([128, 1], F32, tag=f"bias{m}",
                                               name=f"b{path}{m}")
                                nc.sync.dma_start(out=bt[0:mp, :],
                                                  in_=bias_d[m * 128:m * 128 + mp]
                                                  .rearrange("(a b) -> a b", b=1))
                                nc.vector.tensor_scalar_add(a[0:mp, :], ps[0:mp, :],
                                                            bt[0:mp, :])
                            else:
                                nc.scalar.activation(out=a[0:mp, :], in_=ps[0:mp, :],
                                                     func=AF.Copy)
                            acts.append(a)
                            if m < 4:   # latent chunks: accumulate sum of squares
                                sq = p1tmp.tile([128, R], F32R, tag="sq")
                                nc.vector.tensor_mul(sq[:, :], a[:, :], a[:, :])
                                nc.tensor.matmul(sums_ps[:, :], ones_col[:, :],
                                                 sq[:, :],
                                                 start=(m == 0), stop=(m == 3),
                                                 skip_group_check=True)
                        # rstd = 1/sqrt(mean + eps)
                        std = p1tmp.tile([1, R], F32, tag="std")
                        nc.scalar.activation(out=std[:, :], in_=sums_ps[:, :],
                                             func=AF.Sqrt,
                                             scale=1.0 / (QLR if path == 0 else KVLR),
                                             bias=eps_t[:, :])
                        rstd_f = p1tmp.tile([1, R], F32, tag="rstdf")
                        nc.vector.reciprocal(out=rstd_f[:, :], in_=std[:, :])
                        rstd = p1tmp.tile([1, R], F32R, tag="rstd")
                        nc.vector.tensor_copy(out=rstd[:, :], in_=rstd_f[:, :])
                        for m in range(4):
                            rep_ps = p1ps2.tile([128, R], F32, tag="p1rep")
                            nc.tensor.matmul(
                                rep_ps[:, :],
                                nw_t[:, path * QLR + m * 128:
                                     path * QLR + (m + 1) * 128],
                                rstd[:, :],
                                start=True, stop=True)
                            nc.vector.tensor_mul(stage[:, m * R:(m + 1) * R],
                                                 acts[m][:, :], rep_ps[:, :])
                        if path == 1:   # rope on k_pe chunk rows 0:64
                            kpe = acts[4]
                            y = p1tmp.tile([64, R], F32, tag="y1")
                            nc.vector.tensor_mul(y[:, :], kpe[0:64, :], t1c[:, :])
                            sw = p1tmp.tile([64, R], F32, tag="sw1")
                            nc.sync.dma_start(out=sw[0:32, :], in_=kpe[32:64, :])
                            nc.sync.dma_start(out=sw[32:64, :], in_=kpe[0:32, :])
                            z = p1tmp.tile([64, R], F32, tag="z1")
                            nc.vector.tensor_mul(z[:, :], sw[:, :], t1s[:, :])
                            nc.vector.scalar_tensor_tensor(
                                out=stage[0:64, 4 * R:5 * R], in0=z[:, :],
                                scalar=sgn_t[0:64, :], in1=y[:, :],
                                op0=ALU.mult, op1=ALU.add)
                            nc.sync.dma_start(out=agkv_in[:, :], in_=stage[:, :])
                            # fire AG1 as soon as kv payload is stored
                            if sim_mode:
                                nc.sync.dma_start(out=agkv_out[0, 0:1, :],
                                                  in_=agkv_in[0:1, :])
                            else:
                                nc.gpsimd.collective_compute(
                                    "AllGather", ALU.bypass,
                                    replica_groups=[list(range(NC))],
                                    ins=[agkv_in.opt()], outs=[agkv_out.opt()])
                        else:
                            nc.sync.dma_start(out=agq_in[:, :], in_=stage[:, :])
                            if sim_mode:
                                nc.sync.dma_start(out=agq_out[0, 0:1, :],
                                                  in_=agq_in[0:1, :])
                            else:
                                nc.gpsimd.collective_compute(
                                    "AllGather", ALU.bypass,
                                    replica_groups=[list(range(NC))],
                                    ins=[agq_in.opt()], outs=[agq_out.opt()])

                # ============== PHASE 2+3: up-projections + attention ============
                p4w = tc.alloc_tile_pool(name=f"p4w{_rep}", bufs=2)
                wo_n = [None] * 4

                def load_wo(n_):
                    wo_n[n_] = p4w.tile([128, 16 * 512], BF16, tag="wo",
                                        name=f"wo{n_}")
                    nc.sync.dma_start(out=wo_n[n_],
                                      in_=woT[:, n_ * 16 * 512:(n_ + 1) * 16 * 512])
                if flags['wob']:
                    wob_t = p4w.tile([1, DIM], BF16, tag="wob")
                qkv = tc.alloc_tile_pool(name=f"qkv{_rep}", bufs=1)
                q_nope = [qkv.tile([128, N], BF16, name=f"q_nope{i}")
                          for i in range(HPC)]
                q_ropeP = qkv.tile([128, N], BF16)
                k_nope = [qkv.tile([128, N], BF16, name=f"k_nope{i}")
                          for i in range(HPC)]
                k_pe2 = qkv.tile([128, N], BF16)
                vt = qkv.tile([128, N // 128, HPC * VD], BF16)

                with ExitStack() as stk:
                    p2w = stk.enter_context(tc.tile_pool(name=f"p2w{_rep}", bufs=1))
                    p2lat = stk.enter_context(tc.tile_pool(name=f"p2lat{_rep}", bufs=2))
                    p2tmp = stk.enter_context(tc.tile_pool(name=f"p2tmp{_rep}", bufs=1))
                    p3m = stk.enter_context(tc.tile_pool(name=f"p3m{_rep}", bufs=1))
                    p3p = stk.enter_context(tc.tile_pool(name=f"p3p{_rep}", bufs=3))
                    p3o = stk.enter_context(tc.tile_pool(name=f"p3o{_rep}", bufs=2))
                    p2ps = stk.enter_context(
                        tc.tile_pool(name=f"p2ps{_rep}", bufs=2, space="PSUM"))
                    p2psv = stk.enter_context(
                        tc.tile_pool(name=f"p2psv{_rep}", bufs=1, space="PSUM"))
                    p3sc = stk.enter_context(
                        tc.tile_pool(name=f"p3sc{_rep}", bufs=2, space="PSUM"))
                    p3out = stk.enter_context(
                        tc.tile_pool(name=f"p3out{_rep}", bufs=1, space="PSUM"))
                    p3rs = stk.enter_context(
                        tc.tile_pool(name=f"p3rs{_rep}", bufs=1, space="PSUM"))

                    tqc = p2w.tile([128, S], F32)
                    nc.sync.dma_start(out=tqc, in_=trigqc[:, :])
                    tqs = p2w.tile([128, S], F32)
                    nc.sync.dma_start(out=tqs, in_=trigqs[:, :])
                    wqb_t = p2w.tile([128, 3 * 4 * 128], BF16)
                    nc.sync.dma_start(out=wqb_t, in_=wqbT[:, :])
                    wkb_t = p2w.tile([128, 2 * 4 * 128], BF16)
                    nc.sync.dma_start(out=wkb_t, in_=wkbT[:, :])
                    wvb_t = p2w.tile([128, 4 * 256], BF16)
                    nc.sync.dma_start(out=wvb_t, in_=wvbT[:, :])
                    mtiles = [p3m.tile([128, 512], BF16, name=f"mt{i}")
                              for i in range(nmask)]
                    for i in range(nmask):
                        nc.sync.dma_start(out=mtiles[i], in_=maskblk[i, :, :])
                    if flags['bvb']:
                        bvb_t = p2w.tile([1, HPC * VD], BF16)
                        nc.sync.dma_start(out=bvb_t, in_=bvb[:, :])
                    if flags['bqb']:
                        bq_t = [p2w.tile([128, 1], F32, name=f"bqt{m}")
                                for m in range(3)]
                        for m in range(3):
                            nc.sync.dma_start(out=bq_t[m],
                                              in_=bqb[m * 128:(m + 1) * 128]
                                              .rearrange("(a b) -> a b", b=1))
                    if flags['bkb']:
                        bk_t = [p2w.tile([128, 1], F32, name=f"bkt{m}")
                                for m in range(2)]
                        for m in range(2):
                            nc.sync.dma_start(out=bk_t[m],
                                              in_=bkb[m * 128:(m + 1) * 128]
                                              .rearrange("(a b) -> a b", b=1))
                    def p2_block(s):
                        tsl = slice(s * R, (s + 1) * R)
                        pos = (s % (S // R)) * R       # position within batch
                        psl = slice(pos, pos + R)
                        kn = p2lat.tile([128, KVC * R], BF16, tag="kn", name=f"kn{s}")
                        nc.sync.dma_start(out=kn, in_=agkv_out[s, :, :])
                        qn = p2lat.tile([128, 4 * R], BF16, tag="qn", name=f"qn{s}")
                        nc.sync.dma_start(out=qn, in_=agq_out[s, :, :])
                        # k_nope (2 chunks: one per head)
                        for m in range(2):
                            ps = p2ps.tile([128, R], F32, tag="p2acc")
                            for k in range(4):
                                nc.tensor.matmul(
                                    ps[:, :],
                                    wkb_t[:, (m * 4 + k) * 128:(m * 4 + k + 1) * 128],
                                    kn[:, k * R:(k + 1) * R],
                                    start=(k == 0), stop=(k == 3))
                            if flags['bkb']:
                                nc.vector.tensor_scalar_add(k_nope[m][:, tsl],
                                                            ps[:, :], bk_t[m][:, :])
                            elif m == 0:
                                nc.vector.tensor_copy(out=k_nope[m][:, tsl],
                                                      in_=ps[:, :])
                            else:
                                nc.scalar.activation(out=k_nope[m][:, tsl],
                                                     in_=ps[:, :], func=AF.Copy)
                        # v (token-major)
                        for mt in range(4):
                            ps = p2psv.tile([128, HPC * VD], F32, tag="p2v")
                            if flags['bvb']:
                                nc.tensor.matmul(ps[:, :], ones_row_b[:, :],
                                                 bvb_t[:, :], start=True, stop=False)
                            for k in range(4):
                                nc.tensor.matmul(
                                    ps[:, :],
                                    kn[:, k * R + mt * 128:k * R + (mt + 1) * 128],
                                    wvb_t[:, k * 256:(k + 1) * 256],
                                    start=(k == 0 and not flags['bvb']),
                                    stop=(k == 3))
                            if mt % 2 == 0:
                                nc.vector.tensor_copy(out=vt[:, s * 4 + mt, :],
                                                      in_=ps[:, :])
                            else:
                                nc.scalar.activation(out=vt[:, s * 4 + mt, :],
                                                     in_=ps[:, :], func=AF.Copy)
                        # q (2 nope chunks + 1 rope chunk)
                        for m in range(3):
                            ps = p2ps.tile([128, R], F32, tag="p2acc")
                            for k in range(4):
                                nc.tensor.matmul(
                                    ps[:, :],
                                    wqb_t[:, (m * 4 + k) * 128:(m * 4 + k + 1) * 128],
                                    qn[:, k * R:(k + 1) * R],
                                    start=(k == 0), stop=(k == 3))
                            if m < 2:
                                if flags['bqb']:
                                    nc.vector.tensor_scalar_add(q_nope[m][:, tsl],
                                                                ps[:, :], bq_t[m][:, :])
                                elif m == 0:
                                    nc.vector.tensor_copy(out=q_nope[m][:, tsl],
                                                          in_=ps[:, :])
                                else:
                                    nc.scalar.activation(out=q_nope[m][:, tsl],
                                                         in_=ps[:, :], func=AF.Copy)
                            else:
                                src = p2tmp.tile([128, R], F32, tag="rst")
                                if flags['bqb']:
                                    nc.vector.tensor_scalar_add(src[:, :], ps[:, :],
                                                                bq_t[m][:, :])
                                else:
                                    nc.scalar.activation(out=src[:, :], in_=ps[:, :],
                                                         func=AF.Copy)
                                y = p2tmp.tile([128, R], F32, tag="y2")
                                nc.vector.tensor_mul(y[:, :], src[:, :], tqc[:, psl])
                                sw = p2tmp.tile([128, R], F32, tag="sw2")
                                nc.sync.dma_start(out=sw[0:32, :], in_=src[32:64, :])
                                nc.sync.dma_start(out=sw[32:64, :], in_=src[0:32, :])
                                nc.sync.dma_start(out=sw[64:96, :], in_=src[96:128, :])
                                nc.sync.dma_start(out=sw[96:128, :], in_=src[64:96, :])
                                z = p2tmp.tile([128, R], F32, tag="z2")
                                nc.vector.tensor_mul(z[:, :], sw[:, :], tqs[:, psl])
                                nc.vector.scalar_tensor_tensor(
                                    out=q_ropeP[:, tsl], in0=z[:, :],
                                    scalar=sgn_t[:, :], in1=y[:, :],
                                    op0=ALU.mult, op1=ALU.add)
                        # k_pe: rows 0:64 (head copy deferred to dup below)
                        nc.sync.dma_start(out=k_pe2[0:64, tsl],
                                          in_=kn[0:64, 4 * R:5 * R])
                        nc.sync.dma_start(out=k_pe2[64:128, tsl],
                                          in_=kn[0:64, 4 * R:5 * R])

                    def attn(b, qc):
                        for lh in range(HPC):
                            rb = slice(lh * 64, lh * 64 + 64)
                            qsl = slice(b * S + qc * 512, b * S + (qc + 1) * 512)
                            out_ps = p3out.tile([128, 512], F32, tag="outp")
                            rs_ps = p3rs.tile([1, 512], F32, tag="rsp")
                            kbs = [kb for kb in range(S // 128)
                                   if cls[qc][kb] != SKIP]
                            for i, kb in enumerate(kbs):
                                ksl = slice(b * S + kb * 128, b * S + kb * 128 + 128)
                                sc = p3sc.tile([128, 512], F32, tag="sc")
                                nc.tensor.matmul(sc[:, :], k_nope[lh][:, ksl],
                                                 q_nope[lh][:, qsl],
                                                 start=True, stop=False)
                                nc.tensor.matmul(sc[:, :], k_pe2[rb, ksl],
                                                 q_ropeP[rb, qsl],
                                                 start=False, stop=True)
                                P = p3p.tile([128, 512], BF16, tag="P")
                                nc.scalar.activation(out=P[:, :], in_=sc[:, :],
                                                     func=AF.Exp, scale=SCALE)
                                if cls[qc][kb] >= 0:
                                    nc.vector.tensor_mul(P[:, :], P[:, :],
                                                         mtiles[cls[qc][kb]][:, :])
                                last = (i == len(kbs) - 1)
                                nc.tensor.matmul(
                                    out_ps[:, :],
                                    vt[:, b * 16 + kb, lh * VD:(lh + 1) * VD],
                                    P[:, :], start=(i == 0), stop=last,
                                    skip_group_check=True)
                                nc.tensor.matmul(rs_ps[:, :], ones_col_b[:, :],
                                                 P[:, :], start=(i == 0), stop=last,
                                                 skip_group_check=True)
                            inv_f = p3o.tile([1, 512], F32, tag="invf")
                            nc.vector.reciprocal(out=inv_f[:, :], in_=rs_ps[:, :])
                            inv = p3o.tile([1, 512], F32R, tag="inv")
                            nc.vector.tensor_copy(out=inv[:, :], in_=inv_f[:, :])
                            rep_ps = p3sc.tile([128, 512], F32, tag="sc")
                            nc.tensor.matmul(rep_ps[:, :], ones_row[:, :],
                                             inv[:, :],
                                             start=True, stop=True)
                            rep_sb = p3o.tile([128, 512], F32, tag="repsb")
                            nc.scalar.activation(out=rep_sb[:, :], in_=rep_ps[:, :],
                                                 func=AF.Copy)
                            ao = p3o.tile([128, 512], BF16, tag="ao")
                            nc.vector.tensor_mul(ao[:, :], out_ps[:, :], rep_sb[:, :])
                            nc.sync.dma_start(
                                out=a2a_in[b * 4 + qc, lh * VD:(lh + 1) * VD, :],
                                in_=ao[:, :])

                    for s in range(NBLK):
                        p2_block(s)
                        if s == 2:   # prefetch wo n=0 during attention
                            load_wo(0)
                            if flags['wob']:
                                nc.sync.dma_start(out=wob_t, in_=wob[:, :])
                        if s == 4:
                            load_wo(1)
                        attn(s // 4, s % 4)

                # ---- AllToAll ----
                if sim_mode:
                    nc.sync.dma_start(out=a2a_out[0, 0:1, :],
                                      in_=a2a_in[0, 0:1, :])
                else:
                    nc.gpsimd.collective_compute(
                        "AllToAll", ALU.bypass,
                        replica_groups=[list(range(NC))],
                        ins=[a2a_in.opt()], outs=[a2a_out.opt()])

                qkv.release()

                # ================= PHASE 4: output projection =====================
                with ExitStack() as stk4:
                    p4l = stk4.enter_context(tc.tile_pool(name=f"p4l{_rep}", bufs=1))
                    p4o = stk4.enter_context(tc.tile_pool(name=f"p4o{_rep}", bufs=4))
                    p4ps = stk4.enter_context(
                        tc.tile_pool(name=f"p4ps{_rep}", bufs=4, space="PSUM"))
                    lt = [p4l.tile([128, 512], BF16, name=f"lt{k}")
                          for k in range(16)]
                    av = a2a_out[:, :, :].rearrange("c (h p) t -> (c h) p t", h=2)
                    for k in range(16):
                        nc.sync.dma_start(out=lt[k], in_=av[k, :, :])

                    def p4_tile(n_, m):
                        ps = p4ps.tile([128, 512], F32, tag="p4acc")
                        if flags['wob']:
                            nc.tensor.matmul(
                                ps[:, :], ones_row_b[:, :],
                                wob_t[:, n_ * 512:(n_ + 1) * 512],
                                start=True, stop=False)
                        for k in range(16):
                            nc.tensor.matmul(
                                ps[:, :],
                                lt[k][:, m * 128:(m + 1) * 128],
                                wo_n[n_][:, k * 512:(k + 1) * 512],
                                start=(k == 0 and not flags['wob']),
                                stop=(k == 15))
                        ob = p4o.tile([128, 512], F32, tag="ob")
                        if (n_ * 4 + m) % 2 == 0:
                            nc.vector.tensor_copy(out=ob[:, :], in_=ps[:, :])
                        else:
                            nc.scalar.activation(out=ob[:, :], in_=ps[:, :],
                                                 func=AF.Copy)
                        nc.sync.dma_start(
                            out=out_c[m * 128:(m + 1) * 128,
                                      n_ * 512:(n_ + 1) * 512],
                            in_=ob[:, :])

                    load_wo(2)
                    for n_ in range(4):
                        if n_ == 1:
                            load_wo(3)
                        for m in range(4):
                            p4_tile(n_, m)

                p4w.release()

    nc.finalize()
    return nc


_ROPE_PERM = np.concatenate([np.arange(0, ROPE, 2), np.arange(1, ROPE, 2)])

_CACHE = {}


def _bf(a):
    return np.ascontiguousarray(np.asarray(a, np.float32).astype(BF))


def _lhsT_tiles(w, nk, nm, mcols):
    """W^T [nk*128, nm*mcols] -> [128, nm*nk*mcols] with col block (m*nk+k)."""
    t = np.ascontiguousarray(w.T)                       # [in, out]
    t = t.reshape(nk, 128, nm, mcols).transpose(1, 2, 0, 3)  # [128, nm, nk, mcols]
    return np.ascontiguousarray(t.reshape(128, nm * nk * mcols))


def _prep_inputs(inputs):
    """Host-side slicing/permutation -> (schedule key data, per-core in_maps)."""
    x = np.ascontiguousarray(np.asarray(inputs['x'], np.float32).reshape(N, DIM))
    mask = np.asarray(inputs['mask'])
    cls, blocks = _classify_mask(mask)

    cos_t, sin_t = _rope_tables()            # [32, S]
    trigqc = np.concatenate([cos_t, cos_t, cos_t, cos_t], 0)   # [128, S]
    trigqs = np.concatenate([sin_t, sin_t, sin_t, sin_t], 0)
    sgn = np.concatenate([-np.ones(32), np.ones(32), -np.ones(32), np.ones(32)]
                         ).astype(np.float32)[:, None]

    wq_a = np.asarray(inputs['wq_a_w'], np.float32)            # [QLR, DIM]
    wkv_a = np.asarray(inputs['wkv_a_w'], np.float32)          # [KVLR+ROPE, DIM]
    wkv_a_p = np.concatenate([wkv_a[:KVLR], wkv_a[KVLR:][_ROPE_PERM]], 0)
    bkva = np.asarray(inputs['wkv_a_b'], np.float32)
    bkva_p = np.concatenate([bkva[:KVLR], bkva[KVLR:][_ROPE_PERM]], 0)

    wq_b = np.asarray(inputs['wq_b_w'], np.float32).reshape(H, QKD, QLR)
    bq_b = np.asarray(inputs['wq_b_b'], np.float32).reshape(H, QKD)
    wkv_b = np.asarray(inputs['wkv_b_w'], np.float32).reshape(H, NOPE + VD, KVLR)
    bkv_b = np.asarray(inputs['wkv_b_b'], np.float32).reshape(H, NOPE + VD)
    wo = np.asarray(inputs['wo_w'], np.float32)                # [DIM, H*VD]

    wkva_pad = np.concatenate([wkv_a_p, np.zeros((64, DIM), np.float32)], 0)

    # woT: moving layout [128, 4n*16k*512]: block (n*16+k) = wo.T[k-rows, n-cols]
    woTt = np.ascontiguousarray(wo.T)                   # [HV 2048, DIM 2048]
    woTt = woTt.reshape(16, 128, 4, 512).transpose(1, 2, 0, 3)  # [128, 4, 16, 512]
    woTt = np.ascontiguousarray(woTt.reshape(128, 4 * 16 * 512))

    shared = {
        'wqaT': _bf(_lhsT_tiles(wq_a, 16, 4, 128)),
        'bqa': np.asarray(inputs['wq_a_b'], np.float32),
        'wkvaT': _bf(_lhsT_tiles(wkva_pad, 16, 5, 128)),
        'bkva': bkva_p,
        'normw': np.concatenate([np.asarray(inputs['q_norm_w'], np.float32),
                                 np.asarray(inputs['kv_norm_w'], np.float32)]
                                )[None, :],
        'trigqc': np.ascontiguousarray(trigqc),
        'trigqs': np.ascontiguousarray(trigqs),
        'sgn': sgn,
        'woT': _bf(woTt),
        'wob': _bf(np.asarray(inputs['wo_b'], np.float32)[None, :]),
        'maskblk': blocks,
    }

    in_maps = []
    for c in range(NC):
        h0, h1 = 2 * c, 2 * c + 1
        wqb_c = np.concatenate([
            wq_b[h0, :NOPE], wq_b[h1, :NOPE],
            wq_b[h0, NOPE:][_ROPE_PERM], wq_b[h1, NOPE:][_ROPE_PERM]], 0)
        bqb_c = np.concatenate([
            bq_b[h0, :NOPE], bq_b[h1, :NOPE],
            bq_b[h0, NOPE:][_ROPE_PERM], bq_b[h1, NOPE:][_ROPE_PERM]], 0)
        wkb_c = np.concatenate([wkv_b[h0, :NOPE], wkv_b[h1, :NOPE]], 0)
        bkb_c = np.concatenate([bkv_b[h0, :NOPE], bkv_b[h1, :NOPE]], 0)
        wvb_c = np.concatenate([wkv_b[h0, NOPE:], wkv_b[h1, NOPE:]], 0)
        bvb_c = np.concatenate([bkv_b[h0, NOPE:], bkv_b[h1, NOPE:]], 0)
        pos = (c % (S // R)) * R
        cos_c, sin_c = cos_t[:, pos:pos + R], sin_t[:, pos:pos + R]
        # x slab token-major: [128, 16*R], col block k = dim chunk k
        xs = x[c * R:(c + 1) * R].T            # [DIM, R]
        xs = xs.reshape(16, 128, R).transpose(1, 0, 2).reshape(128, 16 * R)
        # wvbT: [128 lat, 4k*256]: block k = wvb_c.T[k*128:(k+1)*128, :]
        wvbt = wvb_c.T.reshape(4, 128, 256).transpose(1, 0, 2).reshape(128, 4 * 256)
        m = dict(shared)
        m.update({
            'x_c': _bf(xs),
            'wqbT': _bf(_lhsT_tiles(wqb_c, 4, 3, 128)),
            'bqb': bqb_c,
            'wkbT': _bf(_lhsT_tiles(wkb_c, 4, 2, 128)),
            'bkb': bkb_c,
            'wvbT': _bf(wvbt),
            'bvb': _bf(bvb_c[None, :]),
            'trig1c': np.ascontiguousarray(np.concatenate([cos_c, cos_c], 0)),
            'trig1s': np.ascontiguousarray(np.concatenate([sin_c, sin_c], 0)),
        })
        in_maps.append(m)
    return cls, in_maps


class _Runner:
    """Compile once, execute many times on the 8 axon-tunneled NeuronCores."""

    def __init__(self, nc):
        import jax
        from jax.experimental.shard_map import shard_map
        from jax.sharding import Mesh, PartitionSpec
        from concourse import bass2jax, mybir as _mybir
        bass2jax.install_neuronx_cc_hook()
        self.jax = jax
        in_names, out_names, out_avals, zero_outs = [], [], [], []
        partition_name = (nc.partition_id_tensor.name
                          if nc.partition_id_tensor else None)
        for alloc in nc.m.functions[0].allocations:
            if not isinstance(alloc, _mybir.MemoryLocationSet):
                continue
            name = alloc.memorylocations[0].name
            if alloc.kind == "ExternalInput":
                if name != partition_name:
                    in_names.append(name)
            elif alloc.kind == "ExternalOutput":
                shape = tuple(alloc.tensor_shape)
                dtype = _mybir.dt.np(alloc.dtype)
                out_names.append(name)
                out_avals.append(jax.core.ShapedArray(shape, dtype))
                zero_outs.append(np.zeros(shape, dtype))
        self.n_params = len(in_names)
        self.in_names = list(in_names)
        self.out_names = out_names
        self.out_avals = out_avals
        self.zero_outs = zero_outs
        all_in = in_names + out_names
        if partition_name is not None:
            all_in.append(partition_name)

        def _body(*args):
            operands = list(args)
            if partition_name is not None:
                operands.append(bass2jax.partition_id_tensor())
            outs = bass2jax._bass_exec_p.bind(
                *operands,
                out_avals=tuple(out_avals),
                in_names=tuple(all_in),
                out_names=tuple(out_names),
                lowering_input_output_aliases=(),
                sim_require_finite=True,
                sim_require_nnan=True,
                nc=nc)
            return tuple(outs)

        devices = jax.devices()[:NC]
        self.mesh = Mesh(np.asarray(devices), ("core",))
        n_out = len(out_names)
        in_specs = (PartitionSpec("core"),) * (self.n_params + n_out)
        out_specs = (PartitionSpec("core"),) * n_out
        donate = tuple(range(self.n_params, self.n_params + n_out))
        self.fn = jax.jit(
            shard_map(_body, mesh=self.mesh, in_specs=in_specs,
                      out_specs=out_specs, check_rep=False),
            donate_argnums=donate, keep_unused=True)

    def concat_inputs(self, in_maps):
        return [np.concatenate([np.asarray(in_maps[c][nm])
                                for c in range(NC)], axis=0)
                for nm in self.in_names]

    def zeros(self):
        return [np.zeros((NC * z.shape[0], *z.shape[1:]), z.dtype)
                for z in self.zero_outs]

    def __call__(self, concat_in, concat_zeros):
        out = self.fn(*concat_in, *concat_zeros)
        return out

    def run(self, in_maps):
        outs = self(self.concat_inputs(in_maps), self.zeros())
        res = []
        for c in range(NC):
            res.append({nm: np.asarray(outs[i]).reshape(NC, *self.out_avals[i].shape)[c]
                        for i, nm in enumerate(self.out_names)})
        return res


def _get_exec(cls, nmask, flags):
    key = (tuple(tuple(r) for r in cls), nmask,
           tuple(flags['ba']), flags['bqb'], flags['bkb'], flags['bvb'],
           flags['wob'])
    if key not in _CACHE:
        nc = _build(cls, nmask, flags)
        _CACHE[key] = _Runner(nc)
    return _CACHE[key]


def kernel(**inputs):
    cls, in_maps = _prep_inputs(inputs)
    nmask = max(len(in_maps[0]['maskblk']), 1)
    flags = {
        'ba': (bool(np.any(inputs['wq_a_b'])), bool(np.any(inputs['wkv_a_b']))),
        'bqb': bool(np.any(inputs['wq_b_b'])),
        'bkb': bool(np.any(np.asarray(inputs['wkv_b_b']).reshape(H, NOPE + VD)[:, :NOPE])),
        'bvb': bool(np.any(np.asarray(inputs['wkv_b_b']).reshape(H, NOPE + VD)[:, NOPE:])),
        'wob': bool(np.any(inputs['wo_b'])),
    }
    runner = _get_exec(cls, nmask, flags)
    results = runner.run(in_maps)
    out = np.concatenate([results[c]["out"] for c in range(NC)], 0)
    return out.reshape(B, S, DIM)
